# revision 14
# baseline (speedup 1.0000x reference)
"""Trainium2 Bass kernel for the sliding-window additive-attention layer.

Reference computation (L=4096, D=H=512, P=16):
    wx = x @ Ww.T                                   [L, H]
    u  = x @ Wu.T  (on zero-padded x)               [L+2P, H]
    for each l, window position w (delta in [-16..16] \\ {0}):
        energy = tanh(wx[l] + u[l+delta])           [H]
        score[l, w] = Wv . energy
    attn = softmax(score, axis=w)
    g[l] = sum_w attn[l, w] * x_pad[l + delta_w]    [L, D]

Key algorithmic points of this implementation:
  * ux of the reference (einsum lwd,hd->lwh) is u[l+delta] - computed once.
  * sequence-parallel over 8 cores: 512 rows each + 16-row halos (host-sliced).
  * all heavy compute in bf16 (VectorE 2x adds, bf16 matmuls), fp32 PSUM.
  * scores accumulate into a PSUM tile [32, L] via M=1 matmuls with Wv.
  * softmax normalization deferred: unnormalized exp(score) weights feed a
    banded matmul against x_halo (with a ones-column appended to get Z), and
    the division by Z happens on the final [L,D] tile.
  * the banded weight matrix A.T[j, l] is materialized via a skewed-stride
    DRAM round trip: rows written contiguously, read back with row stride
    (R-1) which shears the band, then DMA-transposed into lhsT orientation.
"""

import os
import numpy as np
import ml_dtypes

import concourse.bass as bass
import concourse.mybir as mybir
import concourse.tile as tile
from concourse import bacc, bass_utils

BF16 = mybir.dt.bfloat16
F32 = mybir.dt.float32
AF = mybir.ActivationFunctionType

L, D, H, P = 4096, 512, 512, 16
M = 8                 # cores
LLOC = L // M         # 512 rows per core
W = 2 * P             # 32 window positions
R = 160               # Adram row stride (>= 127 + 33)
NHC = H // 128        # 4 h-chunks
NDC = D // 128        # 4 d-chunks
NLC = LLOC // 128     # 4 l-chunks
HALO = LLOC + 2 * P   # 544


def _shift_params(w):
    """window index w (reference order) -> (delta, uT column offset)."""
    delta = w - P if w < P else w - P + 1
    return delta, P + delta


def build_nc() -> bass.Bass:
    nc = bacc.Bacc("TRN2", target_bir_lowering=False, debug=False)

    xT_d = nc.dram_tensor("xT", [128, NDC, HALO], BF16, kind="ExternalInput")
    xh_d = nc.dram_tensor("xh", [128, NLC + 1, D + 1], BF16, kind="ExternalInput")
    wwT_d = nc.dram_tensor("wwT", [128, NDC, H], BF16, kind="ExternalInput")
    wuT_d = nc.dram_tensor("wuT", [128, NDC, H], BF16, kind="ExternalInput")
    # wv_exp[p, hc, w, w'] = Wv[128*hc + p] if w' == w else 0 — a masked-column
    # stationary operand so each score matmul lands on its own PSUM row while
    # keeping the output base partition at 0 (PE quadrant restriction).
    wvT_d = nc.dram_tensor("wvT", [128, NHC, W, W], BF16, kind="ExternalInput")
    eye_d = nc.dram_tensor("eye", [128, 128], BF16, kind="ExternalInput")
    out_d = nc.dram_tensor("out", [128, NLC, D], F32, kind="ExternalOutput")
    adram = nc.dram_tensor("adram", [513 * R], BF16)

    with tile.TileContext(nc) as tc:
        with (
            tc.tile_pool(name="persist", bufs=1) as pp,
            tc.tile_pool(name="pre", bufs=2) as pre_pool,
            tc.tile_pool(name="e", bufs=2) as e_pool,
            tc.tile_pool(name="at", bufs=2) as at_pool,
            tc.tile_pool(name="sc_psum", bufs=1, space="PSUM") as sc_psum,
        ):
            # ---- persistent SBUF tiles + input DMAs ----
            xT_sb = pp.tile([128, NDC, HALO], BF16, tag="xT")
            xh_sb = pp.tile([128, NLC + 1, D + 1], BF16, tag="xh")
            wwT_sb = pp.tile([128, NDC, H], BF16, tag="wwT")
            wuT_sb = pp.tile([128, NDC, H], BF16, tag="wuT")
            wvT_sb = pp.tile([128, NHC, W, W], BF16, tag="wvT")
            eye_sb = pp.tile([128, 128], BF16, tag="eye")
            wxT_sb = pp.tile([128, NHC, LLOC], BF16, tag="wxT")
            uE_sb = pp.tile([128, NHC, HALO], BF16, tag="uE")
            uO_sb = pp.tile([128, NHC, HALO], BF16, tag="uO")
            expE_sb = pp.tile([32, LLOC], BF16, tag="expE")
            explw_sb = pp.tile([128, NLC, W], BF16, tag="explw")
            zeros_sb = pp.tile([128, R], BF16, tag="zeros")
            gout_sb = pp.tile([128, NLC, D], F32, tag="gout")
            rz_sb = pp.tile([128, NLC], F32, tag="rz")

            nc.sync.dma_start(xT_sb[:, :, :], xT_d[:, :, :])
            nc.sync.dma_start(wwT_sb[:, :, :], wwT_d[:, :, :])
            nc.sync.dma_start(wuT_sb[:, :, :], wuT_d[:, :, :])
            nc.sync.dma_start(wvT_sb[:, :, :, :], wvT_d[:, :, :, :])
            nc.scalar.dma_start(xh_sb[:, :, :], xh_d[:, :, :])
            nc.scalar.dma_start(eye_sb[:, :], eye_d[:, :])

            # zero-fill all of Adram (guard row 0, band rows 1..513); the band
            # writes later overwrite cols [0,16) and [17,33) of rows 1..513.
            nc.vector.memset(zeros_sb[:, :], 0.0)
            nc.sync.dma_start(bass.AP(adram, 0, [[1, R]]), zeros_sb[0:1, :])
            for q in range(4):
                nc.sync.dma_start(
                    bass.AP(adram, (1 + 128 * q) * R, [[R, 128], [1, R]]),
                    zeros_sb[:, :],
                )

            # ---- phase 1: wxT[h, l] and uT[h, l'] via PE, cast to bf16 ----
            with tc.tile_pool(name="p1_psum", bufs=2, space="PSUM") as p1_psum:
                for hc in range(NHC):
                    hs = slice(128 * hc, 128 * hc + 128)
                    wx_ps = p1_psum.tile([128, LLOC], F32, tag="wx")
                    for dc in range(NDC):
                        nc.tensor.matmul(
                            wx_ps[:, :],
                            wwT_sb[:, dc, hs],
                            xT_sb[:, dc, P:P + LLOC],
                            start=(dc == 0),
                            stop=(dc == NDC - 1),
                        )
                    nc.vector.tensor_copy(wxT_sb[:, hc, :], wx_ps[:, :])
                    u_ps = p1_psum.tile([128, HALO], F32, tag="u")
                    for dc in range(NDC):
                        nc.tensor.matmul(
                            u_ps[:, 0:512],
                            wuT_sb[:, dc, hs],
                            xT_sb[:, dc, 0:512],
                            start=(dc == 0),
                            stop=(dc == NDC - 1),
                        )
                    for dc in range(NDC):
                        nc.tensor.matmul(
                            u_ps[:, 512:HALO],
                            wuT_sb[:, dc, hs],
                            xT_sb[:, dc, 512:HALO],
                            start=(dc == 0),
                            stop=(dc == NDC - 1),
                        )
                    nc.vector.tensor_copy(uE_sb[:, hc, :], u_ps[:, :])
                    # odd-offset copy so every windowed slice is 4B-aligned
                    nc.vector.tensor_copy(
                        uO_sb[:, hc, 0:HALO - 1], uE_sb[:, hc, 1:HALO]
                    )

                # ---- phase 2: energies + scores ----
                # scores PSUM [32, LLOC], row w; accumulated over h-chunks.
                sc_ps = sc_psum.tile([32, LLOC], F32, tag="sc")
                GRP = 16  # shifts per pre/act/e group
                for hc in range(NHC):
                    for wg in range(W // GRP):
                        pre = pre_pool.tile([128, GRP * LLOC], BF16, tag="pre")
                        for i in range(GRP):
                            w = wg * GRP + i
                            delta, off = _shift_params(w)
                            if off % 2 == 0:
                                src = uE_sb[:, hc, off:off + LLOC]
                            else:
                                src = uO_sb[:, hc, off - 1:off - 1 + LLOC]
                            nc.vector.tensor_add(
                                pre[:, i * LLOC:(i + 1) * LLOC],
                                wxT_sb[:, hc, :],
                                src,
                            )
                        e = e_pool.tile([128, GRP * LLOC], BF16, tag="e")
                        half = GRP * LLOC // 2
                        nc.scalar.activation(e[:, 0:half], pre[:, 0:half], AF.Tanh)
                        nc.scalar.activation(e[:, half:], pre[:, half:], AF.Tanh)
                        for i in range(GRP):
                            w = wg * GRP + i
                            nc.tensor.matmul(
                                sc_ps[:, :],
                                wvT_sb[:, hc, w, :],
                                e[:, i * LLOC:(i + 1) * LLOC],
                                start=(hc == 0 and w == 0),
                                stop=(hc == NHC - 1 and w == W - 1),
                            )

            # ---- phase 3: softmax weights -> banded matmul -> normalize ----
            with tc.tile_pool(name="p3_psum", bufs=2, space="PSUM") as p3_psum:
                # unnormalized softmax weights, w-ordered [32, LLOC]
                nc.scalar.activation(expE_sb[:, :], sc_ps[:, :], AF.Exp)

                # transpose [w, l] -> [l, w] per l-chunk (PE), then write the
                # 33-wide band rows of Adram in two runs, skipping center c=16
                for lc in range(NLC):
                    tp_ps = p3_psum.tile([128, W], BF16, tag="tp")
                    nc.tensor.transpose(
                        tp_ps[:, :],
                        expE_sb[:, 128 * lc:128 * lc + 128],
                        eye_sb[0:32, 0:32],
                    )
                    nc.vector.tensor_copy(explw_sb[:, lc, :], tp_ps[:, :])
                nc.sync.dma_start(
                    bass.AP(adram, R, [[R, 128], [128 * R, NLC], [1, P]]),
                    explw_sb[:, :, 0:P],
                )
                nc.sync.dma_start(
                    bass.AP(adram, R + P + 1, [[R, 128], [128 * R, NLC], [1, P]]),
                    explw_sb[:, :, P:W],
                )

                # skewed transpose-reads: AT[j, l] tiles, then banded matmuls
                for lc in range(NLC):
                    base = (1 + 128 * lc) * R
                    at1 = at_pool.tile([128, 128], BF16, tag="at1")
                    nc.sync.dma_start_transpose(
                        at1[:, :], bass.AP(adram, base, [[R - 1, 128], [1, 128]])
                    )
                    at2 = at_pool.tile([128, 128], BF16, tag="at2")
                    nc.scalar.dma_start_transpose(
                        at2[:, :], bass.AP(adram, base + 128, [[R - 1, 128], [1, 128]])
                    )
                    g_ps = p3_psum.tile([128, D + 1], F32, tag="g")
                    nc.tensor.matmul(
                        g_ps[:, 0:D], at1[:, :], xh_sb[:, lc, 0:D],
                        start=True, stop=False,
                    )
                    nc.tensor.matmul(
                        g_ps[:, 0:D], at2[0:32, :], xh_sb[0:32, lc + 1, 0:D],
                        start=False, stop=True,
                    )
                    nc.tensor.matmul(
                        g_ps[:, D:D + 1], at1[:, :], xh_sb[:, lc, D:D + 1],
                        start=True, stop=False,
                    )
                    nc.tensor.matmul(
                        g_ps[:, D:D + 1], at2[0:32, :], xh_sb[0:32, lc + 1, D:D + 1],
                        start=False, stop=True,
                    )
                    nc.vector.reciprocal(rz_sb[:, lc:lc + 1], g_ps[:, D:D + 1])
                    nc.vector.tensor_scalar_mul(
                        gout_sb[:, lc, :], g_ps[:, 0:D], rz_sb[:, lc:lc + 1]
                    )

            nc.scalar.dma_start(out_d[:, :, :], gout_sb[:, :, :])

    nc.compile()
    return nc


def make_in_maps(x, Ww, Wu, Wv):
    bf = ml_dtypes.bfloat16
    x = np.asarray(x, np.float32)
    x_pad = np.zeros((L + 2 * P, D), np.float32)
    x_pad[P:P + L] = x

    wwT = np.ascontiguousarray(Ww.T).astype(bf).reshape(NDC, 128, H).transpose(1, 0, 2)
    wuT = np.ascontiguousarray(Wu.T).astype(bf).reshape(NDC, 128, H).transpose(1, 0, 2)
    wv_chunks = np.asarray(Wv, np.float32)[0].astype(bf).reshape(NHC, 128)
    wvT = np.zeros((128, NHC, W, W), bf)
    for hc in range(NHC):
        for w in range(W):
            wvT[:, hc, w, w] = wv_chunks[hc]
    eye = np.eye(128, dtype=bf)

    in_maps = []
    for m in range(M):
        xh = x_pad[LLOC * m: LLOC * m + HALO]
        xh_ones = np.concatenate([xh, np.ones((HALO, 1), np.float32)], 1).astype(bf)
        xh_a = np.zeros((128, NLC + 1, D + 1), bf)
        xh_a[:, :NLC] = xh_ones[:512].reshape(NLC, 128, D + 1).transpose(1, 0, 2)
        xh_a[0:32, NLC] = xh_ones[512:HALO]
        xT = np.ascontiguousarray(xh.T).astype(bf)  # [D, 544]
        xT_a = xT.reshape(NDC, 128, HALO).transpose(1, 0, 2)
        in_maps.append({
            "xT": np.ascontiguousarray(xT_a),
            "xh": np.ascontiguousarray(xh_a),
            "wwT": np.ascontiguousarray(wwT),
            "wuT": np.ascontiguousarray(wuT),
            "wvT": np.ascontiguousarray(wvT),
            "eye": eye,
        })
    return in_maps


def assemble_out(results):
    shards = []
    for m in range(M):
        o = np.asarray(results[m]["out"])  # [128, NLC, D]
        shards.append(o.transpose(1, 0, 2).reshape(LLOC, D))
    return np.concatenate(shards, 0).astype(np.float32)


def kernel(x, Ww, Wu, Wv):
    nc = build_nc()
    in_maps = make_in_maps(x, Ww, Wu, Wv)
    res = bass_utils.run_bass_kernel_spmd(nc, in_maps, core_ids=list(range(M)))
    return assemble_out(res.results)


# revision 17
# speedup vs baseline: 1.0185x; 1.0185x over previous
"""Trainium2 Bass kernel for the sliding-window additive-attention layer.

Reference computation (L=4096, D=H=512, P=16):
    wx = x @ Ww.T                                   [L, H]
    u  = x @ Wu.T  (on zero-padded x)               [L+2P, H]
    for each l, window position w (delta in [-16..16] \\ {0}):
        energy = tanh(wx[l] + u[l+delta])           [H]
        score[l, w] = Wv . energy
    attn = softmax(score, axis=w)
    g[l] = sum_w attn[l, w] * x_pad[l + delta_w]    [L, D]

Key algorithmic points of this implementation:
  * ux of the reference (einsum lwd,hd->lwh) is u[l+delta] - computed once.
  * sequence-parallel over 8 cores: 512 rows each + 16-row halos (host-sliced).
  * all heavy compute in bf16 (VectorE 2x adds, bf16 matmuls), fp32 PSUM.
  * pre-activations for 8 shifts at a time in one VectorE op (strided 3-D AP
    over the halo axis, partition-broadcast wx) - amortizes DVE op overhead.
  * scores accumulate into one PSUM tile [32, L]: per (h-chunk, w) a matmul
    whose stationary operand is Wv masked into column w (PE outputs must
    start at partition 0/32/64/96, so M=1 row-placement is not allowed).
  * softmax normalization deferred: unnormalized exp(score) weights feed a
    banded matmul against x_halo; Z comes from a row-sum of the band and the
    division by Z happens on the final [L, D] tile.
  * the banded weight matrix is materialized via a skewed-stride DRAM round
    trip: rows of exp values written contiguously at [l, c], read back with
    row stride (R-1) which shears the band into [l, j] tiles, then
    TensorE-transposed into the [j, l] stationary operand.
"""

import numpy as np
import ml_dtypes

import concourse.bass as bass
import concourse.mybir as mybir
import concourse.tile as tile
from concourse import bacc, bass_utils

BF16 = mybir.dt.bfloat16
F32 = mybir.dt.float32
AF = mybir.ActivationFunctionType

L, D, H, P = 4096, 512, 512, 16
M = 8                 # cores
LLOC = L // M         # 512 rows per core
W = 2 * P             # 32 window positions
R = 160               # Adram row stride (>= 127 + 33)
NHC = H // 128        # 4 h-chunks
NDC = D // 128        # 4 d-chunks
NLC = LLOC // 128     # 4 l-chunks
HALO = LLOC + 2 * P   # 544
GRP = 16              # shifts per pre/act/e group


def _ap3(base, extra_off, mid_step, mid_n, inner_n):
    """[[p_step, 128], [mid_step, mid_n], [1, inner_n]] view of a 2-D slice."""
    p_step = base.ap[0][0]
    return bass.AP(base.tensor, base.offset + extra_off,
                   [[p_step, base.ap[0][1]], [mid_step, mid_n], [1, inner_n]])


def build_nc() -> bass.Bass:
    nc = bacc.Bacc("TRN2", target_bir_lowering=False, debug=False)

    xT_d = nc.dram_tensor("xT", [128, NDC, HALO], BF16, kind="ExternalInput")
    xh_d = nc.dram_tensor("xh", [128, NLC + 1, D], BF16, kind="ExternalInput")
    wwT_d = nc.dram_tensor("wwT", [128, NDC, H], BF16, kind="ExternalInput")
    wuT_d = nc.dram_tensor("wuT", [128, NDC, H], BF16, kind="ExternalInput")
    # wv_exp[p, hc, w, w'] = Wv[128*hc + p] if w' == w else 0
    wvT_d = nc.dram_tensor("wvT", [128, NHC, W, W], BF16, kind="ExternalInput")
    eye_d = nc.dram_tensor("eye", [128, 128], BF16, kind="ExternalInput")
    out_d = nc.dram_tensor("out", [128, NLC, D], F32, kind="ExternalOutput")
    adram = nc.dram_tensor("adram", [513 * R], BF16)

    with tile.TileContext(nc) as tc:
        with (
            tc.tile_pool(name="persist", bufs=1) as pp,
            tc.tile_pool(name="pre", bufs=2) as pre_pool,
            tc.tile_pool(name="e", bufs=2) as e_pool,
            tc.tile_pool(name="ac", bufs=2) as ac_pool,
            tc.tile_pool(name="sc_psum", bufs=1, space="PSUM") as sc_psum,
        ):
            # ---- persistent SBUF tiles + input DMAs ----
            xT_sb = pp.tile([128, NDC, HALO], BF16, tag="xT")
            xh_sb = pp.tile([128, NLC + 1, D], BF16, tag="xh")
            wwT_sb = pp.tile([128, NDC, H], BF16, tag="wwT")
            wuT_sb = pp.tile([128, NDC, H], BF16, tag="wuT")
            wvT_sb = pp.tile([128, NHC, W, W], BF16, tag="wvT")
            eye_sb = pp.tile([128, 128], BF16, tag="eye")
            wxT_sb = pp.tile([128, NHC, LLOC], BF16, tag="wxT")
            uE_sb = pp.tile([128, NHC, HALO], BF16, tag="uE")
            uO_sb = pp.tile([128, NHC, HALO], BF16, tag="uO")
            expE_sb = pp.tile([32, LLOC], BF16, tag="expE")
            explw_sb = pp.tile([128, NLC, W], BF16, tag="explw")
            zeros_sb = pp.tile([128, R], BF16, tag="zeros")
            gout_sb = pp.tile([128, NLC, D], F32, tag="gout")
            z_sb = pp.tile([128, NLC], F32, tag="z")
            rz_sb = pp.tile([128, NLC], F32, tag="rz")

            # phase-1-critical inputs first on the sync queue
            nc.sync.dma_start(xT_sb[:, :, :], xT_d[:, :, :])
            nc.sync.dma_start(wwT_sb[:, :, :], wwT_d[:, :, :])
            nc.sync.dma_start(wuT_sb[:, :, :], wuT_d[:, :, :])
            nc.sync.dma_start(xh_sb[:, :, :], xh_d[:, :, :])
            nc.scalar.dma_start(wvT_sb[:, :, :, :], wvT_d[:, :, :, :])
            nc.scalar.dma_start(eye_sb[:, :], eye_d[:, :])

            # zero-fill all of Adram (guard row 0, band rows 1..513); the band
            # writes later overwrite cols [0,16) and [17,33) of rows 1..513.
            nc.vector.memset(zeros_sb[:, :], 0.0)
            nc.scalar.dma_start(bass.AP(adram, 0, [[1, R]]), zeros_sb[0:1, :])
            for q in range(4):
                nc.scalar.dma_start(
                    bass.AP(adram, (1 + 128 * q) * R, [[R, 128], [1, R]]),
                    zeros_sb[:, :],
                )

            # ---- phase 1: wxT[h, l] and uT[h, l'] via PE, cast to bf16 ----
            with tc.tile_pool(name="p1_psum", bufs=2, space="PSUM") as p1_psum:
                for hc in range(NHC):
                    hs = slice(128 * hc, 128 * hc + 128)
                    wx_ps = p1_psum.tile([128, LLOC], F32, tag="wx")
                    for dc in range(NDC):
                        nc.tensor.matmul(
                            wx_ps[:, :],
                            wwT_sb[:, dc, hs],
                            xT_sb[:, dc, P:P + LLOC],
                            start=(dc == 0),
                            stop=(dc == NDC - 1),
                        )
                    nc.vector.tensor_copy(wxT_sb[:, hc, :], wx_ps[:, :])
                    u_ps = p1_psum.tile([128, HALO], F32, tag="u")
                    for dc in range(NDC):
                        nc.tensor.matmul(
                            u_ps[:, 0:512],
                            wuT_sb[:, dc, hs],
                            xT_sb[:, dc, 0:512],
                            start=(dc == 0),
                            stop=(dc == NDC - 1),
                        )
                    for dc in range(NDC):
                        nc.tensor.matmul(
                            u_ps[:, 512:HALO],
                            wuT_sb[:, dc, hs],
                            xT_sb[:, dc, 512:HALO],
                            start=(dc == 0),
                            stop=(dc == NDC - 1),
                        )
                    nc.vector.tensor_copy(uE_sb[:, hc, :], u_ps[:, :])
                    # odd-offset copy so every windowed slice is 4B-aligned
                    nc.vector.tensor_copy(
                        uO_sb[:, hc, 0:HALO - 1], uE_sb[:, hc, 1:HALO]
                    )

                # ---- phase 2: energies + scores ----
                # scores PSUM [32, LLOC], row w, one accumulation group over
                # all (hc, w) masked-column matmuls.
                sc_ps = sc_psum.tile([32, LLOC], F32, tag="sc")
                for hc in range(NHC):
                    for wg in range(W // GRP):
                        off0 = 0 if wg == 0 else P + 1   # uT col of shift i=0
                        pre = pre_pool.tile([128, GRP * LLOC], BF16, tag="pre")
                        wx_b = wxT_sb[:, hc, :].unsqueeze(1).to_broadcast(
                            [128, GRP // 2, LLOC])
                        for i0 in (0, 1):
                            off = off0 + i0
                            src_t = uE_sb if off % 2 == 0 else uO_sb
                            c0 = off - (off % 2)
                            src = _ap3(src_t[:, hc, 0:LLOC], c0, 2, GRP // 2, LLOC)
                            dst = _ap3(pre[:, 0:LLOC], i0 * LLOC,
                                       2 * LLOC, GRP // 2, LLOC)
                            nc.vector.tensor_add(dst, wx_b, src)
                        e = e_pool.tile([128, GRP * LLOC], BF16, tag="e")
                        half = GRP * LLOC // 2
                        nc.scalar.activation(e[:, 0:half], pre[:, 0:half], AF.Tanh)
                        nc.scalar.activation(e[:, half:], pre[:, half:], AF.Tanh)
                        for i in range(GRP):
                            w = wg * GRP + i
                            nc.tensor.matmul(
                                sc_ps[:, :],
                                wvT_sb[:, hc, w, :],
                                e[:, i * LLOC:(i + 1) * LLOC],
                                start=(hc == 0 and w == 0),
                                stop=(hc == NHC - 1 and w == W - 1),
                            )

            # ---- phase 3: softmax weights -> banded matmul -> normalize ----
            with (
                tc.tile_pool(name="p3s_psum", bufs=4, space="PSUM") as p3s_psum,
                tc.tile_pool(name="p3g_psum", bufs=2, space="PSUM") as p3g_psum,
            ):
                # unnormalized softmax weights, w-ordered [32, LLOC]
                nc.scalar.activation(expE_sb[:, :], sc_ps[:, :], AF.Exp)

                for lc in range(NLC):
                    # transpose [w, l] -> [l, w], write 33-wide band rows of
                    # Adram in two runs (skip center col 16, which stays 0)
                    tp_ps = p3s_psum.tile([128, 128], BF16, tag="tp")
                    nc.tensor.transpose(
                        tp_ps[:, 0:32],
                        expE_sb[:, 128 * lc:128 * lc + 128],
                        eye_sb[0:32, 0:32],
                    )
                    nc.vector.tensor_copy(explw_sb[:, lc, :], tp_ps[:, 0:32])
                    row0 = (1 + 128 * lc) * R
                    nc.scalar.dma_start(
                        bass.AP(adram, row0, [[R, 128], [1, P]]),
                        explw_sb[:, lc, 0:P],
                    )
                    nc.scalar.dma_start(
                        bass.AP(adram, row0 + P + 1, [[R, 128], [1, P]]),
                        explw_sb[:, lc, P:W],
                    )

                for lc in range(NLC):
                    # skewed re-read shears the band: ac[l, jf] = A[l, j]
                    # for j = 128*lc + jf (zero outside the 33-wide window)
                    ac = ac_pool.tile([128, R], BF16, tag="ac")
                    nc.sync.dma_start(
                        ac[:, :],
                        bass.AP(adram, (1 + 128 * lc) * R, [[R - 1, 128], [1, R]]),
                    )
                    nc.vector.tensor_reduce(
                        z_sb[:, lc:lc + 1], ac[:, :],
                        axis=mybir.AxisListType.X, op=mybir.AluOpType.add,
                    )
                    nc.vector.reciprocal(rz_sb[:, lc:lc + 1], z_sb[:, lc:lc + 1])
                    at1_ps = p3s_psum.tile([128, 128], BF16, tag="tp")
                    nc.tensor.transpose(at1_ps[:, :], ac[:, 0:128], eye_sb[:, :])
                    at2_ps = p3s_psum.tile([128, 128], BF16, tag="tp")
                    nc.tensor.transpose(at2_ps[0:32, :], ac[:, 128:R], eye_sb[:, :])
                    at1 = ac_pool.tile([128, 128], BF16, tag="at1s")
                    nc.vector.tensor_copy(at1[:, :], at1_ps[:, :])
                    at2 = ac_pool.tile([32, 128], BF16, tag="at2s")
                    nc.vector.tensor_copy(at2[:, :], at2_ps[0:32, :])

                    g_ps = p3g_psum.tile([128, D], F32, tag="g")
                    nc.tensor.matmul(
                        g_ps[:, :], at1[:, :], xh_sb[:, lc, :],
                        start=True, stop=False,
                    )
                    nc.tensor.matmul(
                        g_ps[:, :], at2[:, :], xh_sb[0:32, lc + 1, :],
                        start=False, stop=True,
                    )
                    nc.vector.tensor_scalar_mul(
                        gout_sb[:, lc, :], g_ps[:, :], rz_sb[:, lc:lc + 1]
                    )
                    nc.scalar.dma_start(out_d[:, lc, :], gout_sb[:, lc, :])

    nc.compile()
    return nc


def make_in_maps(x, Ww, Wu, Wv):
    bf = ml_dtypes.bfloat16
    x = np.asarray(x, np.float32)
    x_pad = np.zeros((L + 2 * P, D), np.float32)
    x_pad[P:P + L] = x

    wwT = np.ascontiguousarray(Ww.T).astype(bf).reshape(NDC, 128, H).transpose(1, 0, 2)
    wuT = np.ascontiguousarray(Wu.T).astype(bf).reshape(NDC, 128, H).transpose(1, 0, 2)
    wv_chunks = np.asarray(Wv, np.float32)[0].astype(bf).reshape(NHC, 128)
    wvT = np.zeros((128, NHC, W, W), bf)
    for hc in range(NHC):
        for w in range(W):
            wvT[:, hc, w, w] = wv_chunks[hc]
    eye = np.eye(128, dtype=bf)

    in_maps = []
    for m in range(M):
        xh = x_pad[LLOC * m: LLOC * m + HALO].astype(bf)       # [544, D]
        xh_a = np.zeros((128, NLC + 1, D), bf)
        xh_a[:, :NLC] = xh[:512].reshape(NLC, 128, D).transpose(1, 0, 2)
        xh_a[0:32, NLC] = xh[512:HALO]
        xT = np.ascontiguousarray(x_pad[LLOC * m: LLOC * m + HALO].T).astype(bf)
        xT_a = xT.reshape(NDC, 128, HALO).transpose(1, 0, 2)
        in_maps.append({
            "xT": np.ascontiguousarray(xT_a),
            "xh": np.ascontiguousarray(xh_a),
            "wwT": np.ascontiguousarray(wwT),
            "wuT": np.ascontiguousarray(wuT),
            "wvT": np.ascontiguousarray(wvT),
            "eye": eye,
        })
    return in_maps


def assemble_out(results):
    shards = []
    for m in range(M):
        o = np.asarray(results[m]["out"]).reshape(128, NLC, D)
        shards.append(o.transpose(1, 0, 2).reshape(LLOC, D))
    return np.concatenate(shards, 0).astype(np.float32)


def kernel(x, Ww, Wu, Wv):
    nc = build_nc()
    in_maps = make_in_maps(x, Ww, Wu, Wv)
    res = bass_utils.run_bass_kernel_spmd(nc, in_maps, core_ids=list(range(M)))
    return assemble_out(res.results)
